# revision 1
# baseline (speedup 1.0000x reference)
"""Trainium2 Bass kernel for per-sample covariance pooling + fc + L2 norm.

Reference computation (per sample b of B=32):
    xc  = x[b] - mean(x[b], axis=0)            # x[b]: [N=20000, D=64]
    cov = xc.T @ xc / (N-1)                    # [64, 64]
    out = normalize(cov.flatten() @ W + b)     # [256]

Kernel formulation (scale/norm invariant):
    G = x.T @ x, s = sum(x, axis=0)            # one PE pass over x
    C = G - s s^T / N                          # = (N-1) * cov
    out = normalize(C.flatten() @ W + b*(N-1)) # identical result

Sharding: data-parallel over batch, 4 samples per core on 8 cores.
W and bias are replicated. x is pre-packed on the host into the SBUF
partition-major layout with a baked-in ones column (gives s for free
as row 64 of the G matmul output) in bf16; W and the feat tile are
fp16 (PSUM accumulation is always fp32). Measured end-to-end rel err
vs the f64 reference: ~3.4e-4. HW exec time ~63-66 us per NEFF run.
"""

import sys

import numpy as np
import ml_dtypes

for _p in ("/opt/trn_rl_repo",):
    if _p not in sys.path:
        sys.path.append(_p)

# Problem shapes (hardcoded per contract).
B, N, D, OUT = 32, 20000, 64, 256
NCORES = 8
BPC = B // NCORES            # samples per core
P = 128                      # SBUF partitions / matmul contraction tile
AUG = D + 1                  # x columns + ones column
NCH = (N + P - 1) // P       # 157 contraction chunks of 128 rows
NPAD = NCH * P               # 20096 rows after zero padding
KC = (D * D) // P            # 32 fc contraction chunks
# DMA schedule: (chunk offset, chunks per DMA); 32 chunks = 532 KB bf16
DMA_TILES = [(0, 32), (32, 32), (64, 32), (96, 32), (128, 29)]
FILL_PER_TILE = 1            # HAM-warming dummy matmuls per x tile

_CACHE = {}


def _split_drain_and_barrier(self, tick_clock, wait_clock):
    """Replacement for TileContext._drain_and_barrier emitting one drain per
    sem wait: this walrus vintage rejects >1 sync-wait per instruction."""
    import bass_rust
    import concourse.mybir as mybir

    drain_bi = self.nc.sync.drain()
    inst = drain_bi.ins
    wait_clock.add_sem_waits(
        drain_bi.ins, bass_rust.ScopedClock({None: tick_clock.global_clock})
    )
    waits = list(inst.sync_info.on_wait) if inst.sync_info else []
    if len(waits) > 1:
        # one pure sem-wait NoOp per extra wait (cheaper than extra drains)
        inst.sync_info = mybir.SyncInfo(on_wait=waits[:1], on_update=[])
        for w in waits[1:]:
            nop = mybir.InstNoOp(
                name=f"tailwait-{w.ant_name}",
                engine=mybir.EngineType.SP,
                sync_info=mybir.SyncInfo(on_wait=[w], on_update=[]),
                bass_nofuse=True,
            )
            self.nc.sync.add_instruction(nop)

    self.nc.all_engine_barrier()
    assert self.sems is not None
    popped = self.nc._tile_sem_poison_stack.pop()
    assert popped is self._sem_poison
    self.nc.clear_and_free_semaphores(list(self.sems.allocated().values()))
    self.nc.all_engine_barrier()


def _build_nc():
    import types

    import concourse.bass as bass
    import concourse.mybir as mybir
    from concourse.tile import TileContext

    dt = mybir.dt
    AF = mybir.ActivationFunctionType
    nc = bass.Bass()

    xin = nc.dram_tensor("xin", [BPC, NCH * AUG * P], dt.bfloat16, kind="ExternalInput")
    win = nc.dram_tensor("win", [P, KC * OUT], dt.float16, kind="ExternalInput")
    # cols 0:OUT: bias; cols OUT:OUT+BPC: ones (same row -- matmul
    # operands must start at partition 0/32/64)
    bin_ = nc.dram_tensor("bin", [1, OUT + BPC], dt.float32, kind="ExternalInput")
    yout = nc.dram_tensor("yout", [BPC, OUT], dt.float32, kind="ExternalOutput")

    # The walrus vintage here supports only ONE sync-wait on data
    # instructions (DMA pseudo ops, TensorCopy, ...). The whole kernel is
    # structured so every emitted instruction needs at most one wait:
    #  - x tiles get one pool slot per DMA (no slot reuse -> 0 waits)
    #  - per-sample psum G tiles are not reused (gpsum bufs=BPC)
    #  - the s row is read/scaled on a single engine (DVE), and all other
    #    cross-engine joins are relayed so same-engine waits merge
    #  - PE "observes" the W/bias DMA lanes early via dummy matmuls and
    #    the bias matmul, so the fc matmuls only wait on DVE.
    tc = TileContext(nc)
    tc._drain_and_barrier = types.MethodType(_split_drain_and_barrier, tc)
    with tc:
        with (
            tc.tile_pool(name="const", bufs=1) as cpool,
            tc.tile_pool(name="xp", bufs=len(DMA_TILES) * BPC) as xpool,
            tc.tile_pool(name="small", bufs=2) as spool,
            tc.tile_pool(name="featp", bufs=1) as fpool,
            tc.tile_pool(name="gpsum", bufs=BPC, space="PSUM") as gpool,
            tc.tile_pool(name="rpsum", bufs=2, space="PSUM") as rpool,
            tc.tile_pool(name="opsum", bufs=1, space="PSUM") as opool,
        ):
            # DMAs ride the two HWDGE rings (SP + ACT), which each drain
            # strictly in issue order: a monolithic 2 MB W DMA would stall
            # later x tiles behind it, so W is split into 8 slices issued on
            # ring 1 between the x tiles of sample 1.
            w_sb = cpool.tile([P, KC * OUT], dt.float16)
            bias_sb = cpool.tile([1, OUT + BPC], dt.float32)
            nc.sync.dma_start(out=bias_sb[:], in_=bin_[:])

            ring = [nc.sync, nc.scalar]
            rr = [0]

            def ring_dma(out, in_, force=None):
                r = force if force is not None else rr[0] % 2
                if force is None:
                    rr[0] += 1
                ring[r].dma_start(out=out, in_=in_)

            WSLICES = 8
            WSL = KC * OUT // WSLICES
            wq = list(range(WSLICES))  # pending W slice ids

            def issue_w_slices(k):
                for _ in range(k):
                    if wq:
                        c = wq.pop(0)
                        ring_dma(
                            w_sb[:, c * WSL : (c + 1) * WSL],
                            win[:, c * WSL : (c + 1) * WSL],
                            force=1,
                        )

            # feat_sb[p, c, bb] = flattened C for sample bb, fc-chunk layout:
            # element k = c*128 + p of C.flatten(). Using C's symmetry,
            # k = d*64+e maps to (p = (d%2)*64 + e, c = d//2), i.e. chunk c
            # stacks C[:, 2c] (even col) on partitions 0:64 and C[:, 2c+1]
            # (odd col) on partitions 64:128 -- no transpose needed.
            feat_sb = fpool.tile([P, KC, BPC], dt.float16)

            po = opool.tile([BPC, OUT], dt.float32)
            pdum = opool.tile([1, 512], dt.float32, tag="pdum")

            # Pre-warm the PE clock gate (HAM) with dummy matmuls on a memset
            # tile while the first x tile is still in flight: the gate needs
            # ~3.4 us of sustained activity to lift the 1.2 GHz cold throttle.
            dumsrc = cpool.tile([P, 512], dt.bfloat16)
            nc.vector.memset(dumsrc[:], 0.5)
            for _ in range(8):
                nc.tensor.matmul(
                    pdum[:], lhsT=dumsrc[:, 0:1], rhs=dumsrc[:, 0:512],
                    start=True, stop=True,
                )

            def do_sample(bb):
                pg = gpool.tile([AUG, AUG], dt.float32, tag="pg")
                for (i0, nblk) in DMA_TILES:
                    xt = xpool.tile([P, nblk * AUG], dt.bfloat16, tag="xt")
                    # sample 0 serial on ring 0: halving bandwidth across two
                    # rings would delay the first tile (and PE start) by ~1.5us
                    ring_dma(
                        xt[:],
                        xin[bb, i0 * AUG * P : (i0 + nblk) * AUG * P].rearrange(
                            "(p f) -> p f", p=P
                        ),
                        force=0 if bb == 0 else None,
                    )
                    if bb == 1:
                        issue_w_slices(2)
                    for j in range(nblk):
                        ch = xt[:, j * AUG : (j + 1) * AUG]
                        nc.tensor.matmul(
                            pg[:],
                            lhsT=ch,
                            rhs=ch,
                            start=(i0 + j == 0),
                            stop=(i0 + j == NCH - 1),
                        )
                    # HAM-warming filler: DMA delivers tiles slower than the
                    # PE consumes them; idle gaps re-throttle the PE clock to
                    # 1.2 GHz (and cold matmuls run 2x slow). Burn the slack
                    # on wide dummy matmuls over the already-resident tile --
                    # no new deps, keeps the activity monitor at 8/8.
                    for _ in range(FILL_PER_TILE):
                        nc.tensor.matmul(
                            pdum[:], lhsT=xt[:, 0:1], rhs=xt[:, 0:512],
                            start=True, stop=True,
                        )
                # pg[0:64, 0:64] = G, pg[64, 0:64] = s (from the ones column).
                # R = (s/N) s^T into its own psum, relay-copied to SBUF, then
                # feat = G - R fused into the (strided) feat_sb copies. All
                # cross-engine joins funnel through DVE ticks so each
                # instruction needs at most one sync wait (walrus limit).
                s_pos = spool.tile([1, D], dt.float32, tag="spos")
                s_scl = spool.tile([1, D], dt.float32, tag="sscl")
                nc.vector.tensor_copy(s_pos[:], pg[D : D + 1, 0:D])
                nc.vector.tensor_scalar_mul(
                    s_scl[:], pg[D : D + 1, 0:D], 1.0 / (N * (N - 1.0))
                )
                rps = rpool.tile([D, D], dt.float32, tag="rps")
                nc.tensor.matmul(
                    rps[:], lhsT=s_scl[:], rhs=s_pos[:], start=True, stop=True
                )
                rsb = spool.tile([D, D], dt.float32, tag="rsb")
                nc.vector.tensor_copy(rsb[:], rps[:])
                ge = pg[0:D, 0:D].rearrange("p (c two) -> p c two", two=2)
                re = rsb[:].rearrange("p (c two) -> p c two", two=2)
                # feat = G/(N-1) - s s^T/(N(N-1))  (= cov), cast to fp16
                nc.vector.scalar_tensor_tensor(
                    feat_sb[0:D, :, bb], ge[:, :, 0], 1.0 / (N - 1.0),
                    re[:, :, 0], op0=mybir.AluOpType.mult,
                    op1=mybir.AluOpType.subtract,
                )
                nc.vector.scalar_tensor_tensor(
                    feat_sb[D:P, :, bb], ge[:, :, 1], 1.0 / (N - 1.0),
                    re[:, :, 1], op0=mybir.AluOpType.mult,
                    op1=mybir.AluOpType.subtract,
                )
                # keep the PE array warm across the sample-boundary stall
                # (s-row readback -> outer product -> feat writeback chain)
                for _ in range(0 if bb == 0 else 2):
                    nc.tensor.matmul(
                        pdum[:], lhsT=xt[:, 0:1], rhs=xt[:, 0:512],
                        start=True, stop=True,
                    )

            do_sample(0)
            do_sample(1)
            do_sample(2)
            issue_w_slices(WSLICES)  # any stragglers
            # PE observes every W slice's DMA lane (all slices complete
            # during sample 2's stream; no PE stall here) so the fc matmuls
            # later need no DMA waits of their own.
            for c in range(WSLICES):
                nc.tensor.matmul(
                    pdum[0:1, 0:1], lhsT=w_sb[0:1, c * WSL : c * WSL + 1],
                    rhs=w_sb[0:1, c * WSL : c * WSL + 1],
                    start=True, stop=True,
                )
            # Open the fc accumulation with the bias row: po = 1 * bias'.
            nc.tensor.matmul(
                po[:], lhsT=bias_sb[0:1, OUT : OUT + BPC], rhs=bias_sb[0:1, 0:OUT],
                start=True, stop=False,
            )
            do_sample(3)

            # fc: out[bb, o] = bias'[o] + sum_k feat[k, bb] * W[k, o]
            for c in range(KC):
                nc.tensor.matmul(
                    po[:],
                    lhsT=feat_sb[:, c, :],
                    rhs=w_sb[:, c * OUT : (c + 1) * OUT],
                    start=False,
                    stop=(c == KC - 1),
                )

            # L2 normalize rows: out = po / max(||po||, 1e-12). DVE-centric
            # (ACT only for the tiny sqrt -- ACT ops are slow on small tiles).
            posb = spool.tile([BPC, OUT], dt.float32, tag="posb")
            nc.vector.tensor_copy(posb[:], po[:])
            sq = spool.tile([BPC, OUT], dt.float32, tag="sq")
            ss = spool.tile([BPC, 1], dt.float32, tag="ss")
            nc.vector.tensor_mul(sq[:], posb[:], posb[:])
            nc.vector.tensor_reduce(
                ss[:], sq[:], axis=mybir.AxisListType.X, op=mybir.AluOpType.add
            )
            nc.vector.tensor_scalar_max(ss[:], ss[:], 1e-24)
            nrm = spool.tile([BPC, 1], dt.float32, tag="nrm")
            nc.scalar.activation(nrm[:], ss[:], AF.Sqrt)
            inv = spool.tile([BPC, 1], dt.float32, tag="inv")
            nc.vector.reciprocal(inv[:], nrm[:])
            out_sb = spool.tile([BPC, OUT], dt.float32, tag="osb")
            nc.vector.tensor_scalar_mul(out_sb[:], posb[:], inv[:])
            nc.gpsimd.dma_start(out=yout[:], in_=out_sb[:])

    return nc


def _get_nc():
    if "nc" not in _CACHE:
        _CACHE["nc"] = _build_nc()
    return _CACHE["nc"]


def _pack_inputs(x, W, b):
    x = np.asarray(x, dtype=np.float32)
    W = np.asarray(W, dtype=np.float32)
    b = np.asarray(b, dtype=np.float32)

    aug = np.zeros((B, NPAD, AUG), dtype=ml_dtypes.bfloat16)
    aug[:, :N, :D] = x.astype(ml_dtypes.bfloat16)
    aug[:, :, D] = 1.0
    # row n = chunk i*128 + partition p -> [B, p, i, AUG], then regroup into
    # DMA tiles so each dma_start reads one fully contiguous DRAM extent:
    # [B][tile][p][nblk*AUG]
    augT = aug.reshape(B, NCH, P, AUG).transpose(0, 2, 1, 3)  # [B,P,NCH,AUG]
    parts = []
    for (i0, nblk) in DMA_TILES:
        blk = augT[:, :, i0 : i0 + nblk, :].reshape(B, P, nblk * AUG)
        parts.append(blk.reshape(B, P * nblk * AUG))
    augT = np.ascontiguousarray(np.concatenate(parts, axis=1))

    wp = np.ascontiguousarray(
        W.reshape(KC, P, OUT).transpose(1, 0, 2)
    ).reshape(P, KC * OUT).astype(np.float16)
    bp = np.concatenate([b, np.ones(BPC, np.float32)]).reshape(1, OUT + BPC)

    return [
        {
            "xin": np.ascontiguousarray(augT[c * BPC : (c + 1) * BPC]),
            "win": wp,
            "bin": bp,
        }
        for c in range(NCORES)
    ]


def run(x, W, b, trace=False):
    from concourse.bass_utils import run_bass_kernel_spmd

    nc = _get_nc()
    in_maps = _pack_inputs(x, W, b)
    res = run_bass_kernel_spmd(nc, in_maps, list(range(NCORES)), trace=trace)
    out = np.concatenate(
        [res.results[c]["yout"] for c in range(NCORES)], axis=0
    ).astype(np.float32)
    return out, res


def kernel(x, W, b):
    out, _ = run(x, W, b, trace=False)
    return out

